# revision 29
# baseline (speedup 1.0000x reference)
"""Trainium2 Bass kernel for BottleneckedEnsembleAttention (sparse/compacted).

Sharding: 8 cores, core c handles heads [2c, 2c+1] for both batches
(4 independent (b, head) attention problems per core).

Sparsity: the reference zeroes output rows for inactive queries, masks
inactive keys out of the softmax, and inactive tokens never otherwise
contribute.  The host therefore COMPACTS each (b, h) problem to its active
tokens (order-preserving, so the causal mask stays lower-triangular), pads
to NA = ceil(max_active/128)*128, and scatters the device output back into
a zero tensor.  Seed-0 counts are ~1024 of 2048, so NA = 1152: projections
shrink ~2x and attention area ~3.2x.

Host also pre-transposes the compacted X to [HIDDEN, NA] fp16 (no on-device
transposes for X), folds the softmax scale into Wq, and computes compacted
YaRN cos/sin tables [32, NA] fp16 (rows are 32-periodic on device).

Per (b, h) on-device pipeline (all heavy matmuls 1 cycle/col):
  1. qk pass: psum_qk = [Wq*scale | Wk]^T X^T  -> [128, ch] (q^T rows 0-63,
     k^T rows 64-127), fp16 inputs.
  2. RoPE on PE: ev_c = psum_qk * cos, ev_s = psum_qk * sin (DVE);
     cos/sin rows are 32-periodic so rotate-half/q-k-swap permutations
     commute with the elementwise multiplies:
       qkrot = P_rot @ ev_s + ev_c          (2 matmuls into one psum)
       kq    = (Psw P_rot) @ ev_s + Psw @ ev_c   (k in rows 0-63, q in 64-127)
  3. v^T pass (fp16) -> vt [65, NA] with row 64 = active-indicator (for the
     free softmax denominator), PE-transposed to vn [s, 65].
  4. per t-chunk (384 cols): scores^T[s-tile, t] = k lhsT @ q rhs; causal
     mask added INSIDE the matmul via an fp16 strict-upper -60000 triangular
     lhsT against an identity rhs on the diagonal 128-block; exp via ACT;
     att^T[u, t] accumulated over s-tiles (row 64 = denominator);
     o_proj: out[t-tile, 1024] = att^T lhsT @ Wo rhs, scaled by 1/denom at
     PSUM eviction (denom reciprocal as a row, PE-transposed to columns).
  5. store compacted fp16 output rows; host upcasts and scatters.

The emission is software-pipelined across the 4 (b, h) problems: the next
pair's projections/RoPE/v are interleaved as PE gap-filler inside the
current pair's attention chunks, and each chunk's o_proj/store is deferred
one chunk so the PE never waits on eviction chains.  PSUM evictions are
spread across ACT/DVE/Pool engines.
"""

import math
from contextlib import ExitStack

import numpy as np

import concourse.bass as bass
import concourse.mybir as mybir
import concourse.tile as tile
from concourse import bacc
from concourse.bass_utils import run_bass_kernel_spmd

# model constants (must match reference.py)
HIDDEN = 1024
HEADS = 16
HEAD_DIM = 64
THETA = 10000.0
TRAIN_LEN = 2048
SCALE = 4.0
ALPHA = 1.0
BETA = 32.0
B, T = 2, 2048

NCORES = 8
HPC = HEADS // NCORES  # heads per core = 2
NPAIRS = B * HPC       # independent (b, h) problems per core = 4

F32 = mybir.dt.float32
F32R = mybir.dt.float32r
F16 = mybir.dt.float16
BF16 = mybir.dt.bfloat16

ND = HIDDEN // 128  # 8 d-chunks
CW = 384            # chunk width (3 t-tiles)
TPC = CW // 128     # t-tiles per chunk = 3
NEG_TRI = -60000.0  # fp16-representable; exp(score + NEG_TRI) == 0.0


def _yarn_inv_freq():
    half = HEAD_DIM // 2
    pos_freqs = THETA ** (np.arange(half, dtype=np.float32) * 2.0 / HEAD_DIM)
    inv_freq_extra = (1.0 / pos_freqs).astype(np.float32)
    inv_freq_inter = (1.0 / (SCALE * pos_freqs)).astype(np.float32)

    def find_dim(num_rot):
        return (HEAD_DIM * math.log(TRAIN_LEN / (num_rot * 2.0 * math.pi))) / (
            2.0 * math.log(THETA)
        )

    low = max(math.floor(find_dim(BETA)), 0)
    high = min(math.ceil(find_dim(ALPHA)), half - 1)
    ramp = np.clip(
        (np.arange(half, dtype=np.float32) - low) / max(high - low, 1e-3), 0.0, 1.0
    ).astype(np.float32)
    extrap = (1.0 - ramp).astype(np.float32)
    inv_freq = inv_freq_inter * (1.0 - extrap) + inv_freq_extra * extrap
    mscale = 0.1 * math.log(SCALE) + 1.0 if SCALE > 1.0 else 1.0
    return inv_freq.astype(np.float32), np.float32(mscale)


def _perm_consts():
    """Permutation lhsT matrices for RoPE on the PE.

    P_rot: within each 64-row block (q rows 0-63, k rows 64-127),
      (P v)[u] = -v[u+32] for u<32, +v[u-32] for u>=32  (rotate-half w/ sign)
    P_swap: (P v)[u] = v[(u+64) % 128]                  (q<->k block swap)
    Matmul computes lhsT.T @ rhs, so pass the TRANSPOSE of each matrix.
    """
    P_rot = np.zeros((128, 128), dtype=np.float32)
    for blk in (0, 64):
        for u in range(32):
            P_rot[blk + u, blk + u + 32] = -1.0
            P_rot[blk + u + 32, blk + u] = 1.0
    P_swap = np.zeros((128, 128), dtype=np.float32)
    for u in range(128):
        P_swap[u, (u + 64) % 128] = 1.0
    P_swrot = P_swap @ P_rot
    ident = np.eye(128, dtype=np.float32)
    ones = np.ones((128, 128), dtype=np.float32)
    # [5, 128, 128]: lhsT variants (transposed), identity, all-ones
    perms = np.stack(
        [P_rot.T, P_swrot.T, P_swap.T, ident, ones], axis=0
    )
    return np.ascontiguousarray(perms)


def _tri_consts():
    """fp16 [3, 128, 128]: slot 0 = M^T where M[s,t] = NEG_TRI for s > t
    (strict lower triangle in (s, t)), slot 1 = identity, slot 2 = all
    NEG_TRI (for fully-masked s>t blocks left of the diagonal)."""
    M = np.tril(np.full((128, 128), NEG_TRI, dtype=np.float32), k=-1)
    full = np.full((128, 128), NEG_TRI, dtype=np.float32)
    out = np.stack([M.T, np.eye(128, dtype=np.float32), full], axis=0)
    return np.ascontiguousarray(out.astype(np.float16))


def _host_prep(inputs):
    x = np.asarray(inputs["packed_embeddings"], dtype=np.float32)
    pos = np.asarray(inputs["position_ids"])
    act = np.asarray(inputs["active_mask"])
    wq = np.asarray(inputs["q_proj"], dtype=np.float32)
    wk = np.asarray(inputs["k_proj"], dtype=np.float32)
    wv = np.asarray(inputs["v_proj"], dtype=np.float32)
    wo = np.asarray(inputs["o_proj"], dtype=np.float32)

    inv_freq, mscale = _yarn_inv_freq()
    scale = np.float32(mscale / math.sqrt(HEAD_DIM))

    counts = act.sum(axis=-1)  # (B, HEADS)
    nt_act = max(1, int(-(-counts.max() // 128)))
    na = nt_act * 128
    # round tiles up to a multiple of TPC so chunks are uniform
    nt_act = -(-nt_act // TPC) * TPC
    na = nt_act * 128

    idx = [[np.nonzero(act[b, l])[0] for l in range(HEADS)] for b in range(B)]

    xt = np.zeros((B, HEADS, HIDDEN, na), dtype=np.float16)
    cs = np.zeros((B, HEADS, 2, HEAD_DIM // 2, na), dtype=np.float16)
    a01 = np.zeros((B, HEADS, 1, na), dtype=np.float32)
    for b in range(B):
        for l in range(HEADS):
            ii = idx[b][l]
            n = len(ii)
            xt[b, l, :, :n] = x[b, l, ii, :].T
            ang = pos[b, l, ii].astype(np.float32)[:, None] * inv_freq  # (n, 32)
            cs[b, l, 0, :, :n] = np.cos(ang).T
            cs[b, l, 1, :, :n] = np.sin(ang).T
            a01[b, l, 0, :n] = 1.0

    wqk = np.concatenate([wq * scale, wk], axis=-1)  # (L, 1024, 128)
    wqk16 = np.ascontiguousarray(wqk.astype(np.float16))
    wv16 = np.ascontiguousarray(wv.astype(np.float16))
    wo32 = np.ascontiguousarray(wo)

    perms = _perm_consts()
    tri = _tri_consts()
    meta = {"na": na, "nt_act": nt_act, "counts": counts, "idx": idx}
    return xt, cs, a01, wqk16, wv16, wo32, perms, tri, meta


def _build_program(na):
    nt = na // 128
    nc = bacc.Bacc("TRN2", target_bir_lowering=False, debug=False)

    xt_d = nc.declare_dram_parameter("xt", [B, HPC, HIDDEN, na], F16, isOutput=False)
    cs_d = nc.declare_dram_parameter("cs", [B, HPC, 2, HEAD_DIM // 2, na], F16,
                                     isOutput=False)
    a01_d = nc.declare_dram_parameter("a01", [B, HPC, 1, na], F32R, isOutput=False)
    wqk_d = nc.declare_dram_parameter("wqk", [HPC, HIDDEN, 128], F16, isOutput=False)
    wv_d = nc.declare_dram_parameter("wv", [HPC, HIDDEN, HEAD_DIM], F16,
                                     isOutput=False)
    wo_d = nc.declare_dram_parameter("wo", [HPC, HEAD_DIM, HIDDEN], F32R,
                                     isOutput=False)
    perm_d = nc.declare_dram_parameter("perm", [5, 128, 128], F32R, isOutput=False)
    tri_d = nc.declare_dram_parameter("tri", [3, 128, 128], F16, isOutput=False)
    out_d = nc.declare_dram_parameter("out", [B, HPC, na, HIDDEN], F16, isOutput=True)

    with ExitStack() as ctx:
        tc = ctx.enter_context(tile.TileContext(nc))
        _emit(ctx, tc, nc, na, nt, xt_d, cs_d, a01_d, wqk_d, wv_d, wo_d,
              perm_d, tri_d, out_d)
    nc.compile()
    return nc


def _emit(ctx, tc, nc, na, nt, xt_d, cs_d, a01_d, wqk_d, wv_d, wo_d,
          perm_d, tri_d, out_d):
    nchunks = nt // TPC

    # ---- pools ----
    consts = ctx.enter_context(tc.tile_pool(name="consts", bufs=1))
    wpool = ctx.enter_context(tc.tile_pool(name="wpool", bufs=2))
    xtp = ctx.enter_context(tc.tile_pool(name="xt", bufs=2))
    cssp = ctx.enter_context(tc.tile_pool(name="css", bufs=2))
    qkp = ctx.enter_context(tc.tile_pool(name="qk", bufs=2))
    evp = ctx.enter_context(tc.tile_pool(name="ev", bufs=2))
    vtp = ctx.enter_context(tc.tile_pool(name="vt", bufs=2))
    vnp = ctx.enter_context(tc.tile_pool(name="vn", bufs=2))
    ptp = ctx.enter_context(tc.tile_pool(name="pt", bufs=4))
    attp = ctx.enter_context(tc.tile_pool(name="att", bufs=2))
    rap = ctx.enter_context(tc.tile_pool(name="ra", bufs=2))
    outp = ctx.enter_context(tc.tile_pool(name="outsb", bufs=2))

    ps_proj = ctx.enter_context(tc.tile_pool(name="ps_proj", bufs=2, space="PSUM"))
    ps_rk = ctx.enter_context(tc.tile_pool(name="ps_rk", bufs=1, space="PSUM"))
    ps_sc = ctx.enter_context(tc.tile_pool(name="ps_sc", bufs=2, space="PSUM"))
    ps_att = ctx.enter_context(tc.tile_pool(name="ps_att", bufs=1, space="PSUM"))
    ps_o = ctx.enter_context(tc.tile_pool(name="ps_o", bufs=2, space="PSUM"))

    # ---- constants (once) ----
    perm_sb = consts.tile([128, 5, 128], F32R)
    nc.sync.dma_start(out=perm_sb, in_=perm_d.rearrange("k p m -> p k m"))
    tri_sb = consts.tile([128, 3, 128], F16)
    nc.sync.dma_start(out=tri_sb, in_=tri_d.rearrange("k p m -> p k m"))
    ones_sb = consts.tile([128, 1], F32)
    nc.vector.memset(ones_sb, 1.0)

    pairs = [(b, h) for b in range(B) for h in range(HPC)]
    st = {}       # per-pair state
    pending = []  # deferred chunk finishers

    # ---------- phase emitters ----------
    def emit_tables(idx):
        b, h = pairs[idx]
        s = st[idx] = {}
        t_qk = wpool.tile([128, ND, 128], F16, tag="wqk", name="t_qk")
        nc.sync.dma_start(out=t_qk, in_=wqk_d[h].rearrange("(c p) m -> p c m", p=128))
        s["xt"] = xtp.tile([128, ND, na], F16, tag="xt", name="xt_sb")
        nparts = 4 if idx == 0 else 2
        step = ND // nparts
        src = xt_d[b, h].rearrange("(c p) t -> p c t", p=128)
        for j in range(nparts):
            nc.sync.dma_start(out=s["xt"][:, j * step:(j + 1) * step, :],
                              in_=src[:, j * step:(j + 1) * step, :])
        s["cos"] = cssp.tile([128, na], F16, tag="cos", name="cos_sb")
        base = cs_d[b, h, 0]
        nc.sync.dma_start(out=s["cos"], in_=bass.AP(
            tensor=base.tensor, offset=base.offset, ap=[[0, 4]] + list(base.ap)))
        s["sin"] = cssp.tile([128, na], F16, tag="sin", name="sin_sb")
        base = cs_d[b, h, 1]
        nc.sync.dma_start(out=s["sin"], in_=bass.AP(
            tensor=base.tensor, offset=base.offset, ap=[[0, 4]] + list(base.ap)))
        t_v = wpool.tile([128, ND, HEAD_DIM], F16, tag="wv", name="t_v")
        nc.sync.dma_start(out=t_v, in_=wv_d[h].rearrange("(c p) m -> p c m", p=128))
        t_o = wpool.tile([HEAD_DIM, HIDDEN], F32R, tag="wo", name="t_o")
        nc.sync.dma_start(out=t_o, in_=wo_d[h])
        s["wqk"], s["wv"], s["wo"] = t_qk, t_v, t_o
        # vt with active-indicator row 64 (free softmax denominator)
        s["vt"] = vtp.tile([HEAD_DIM + 1, na], F32, tag="vt", name="vt_sb")
        nc.sync.dma_start(out=s["vt"][HEAD_DIM:HEAD_DIM + 1, :]
                          .bitcast(F32R), in_=a01_d[b, h])
        s["a01"] = s["vt"]

    def emit_b_steps(idx):
        # projections + RoPE + v for pair idx, software-pipelined across its
        # chunks so PE never waits on the DVE/ACT RoPE evicts.
        s = st[idx]
        xt, wqk, wv = s["xt"], s["wqk"], s["wv"]
        cos, sin = s["cos"], s["sin"]
        qkrot = qkp.tile([128, na], F32R, tag="qkrot", name="qkrot")
        kq = qkp.tile([128, na], F32R, tag="kq", name="kq")
        s["qkrot"], s["kq"] = qkrot, kq
        vt = s["vt"]
        vcols = HEAD_DIM + 1
        vn = vnp.tile([128, nt * vcols], BF16, tag="vn", name="vn")
        s["vn"] = vn

        pq = {}
        ev = {}

        def qk_mms(cx):
            tsl = slice(cx * CW, (cx + 1) * CW)
            pq[cx] = ps_proj.tile([128, CW], F32, tag="proj", name="pq")
            for dc in range(ND):
                nc.tensor.matmul(pq[cx], lhsT=wqk[:, dc, :], rhs=xt[:, dc, tsl],
                                 start=(dc == 0), stop=(dc == ND - 1))

        def ev_muls(cx):
            tsl = slice(cx * CW, (cx + 1) * CW)
            ev_c = evp.tile([128, CW], F32R, tag="evc", name="ev_c")
            nc.vector.tensor_mul(ev_c, pq[cx], cos[:, tsl])
            ev_s = evp.tile([128, CW], F32R, tag="evs", name="ev_s")
            nc.vector.tensor_mul(ev_s, pq[cx], sin[:, tsl])
            ev[cx] = (ev_c, ev_s)

        def rot_block(cx):
            tsl = slice(cx * CW, (cx + 1) * CW)
            ev_c, ev_s = ev[cx]
            pr = ps_rk.tile([128, CW], F32, tag="rk", name="pr")
            nc.tensor.matmul(pr, lhsT=perm_sb[:, 0, :], rhs=ev_s,
                             start=True, stop=False)
            nc.tensor.matmul(pr, lhsT=perm_sb[:, 3, :], rhs=ev_c,
                             start=False, stop=True, skip_group_check=True)
            nc.vector.tensor_copy(qkrot[:, tsl], pr)

        def kq_block(cx):
            tsl = slice(cx * CW, (cx + 1) * CW)
            ev_c, ev_s = ev[cx]
            pk = ps_rk.tile([128, CW], F32, tag="rk", name="pk")
            nc.tensor.matmul(pk, lhsT=perm_sb[:, 1, :], rhs=ev_s,
                             start=True, stop=False)
            nc.tensor.matmul(pk, lhsT=perm_sb[:, 2, :], rhs=ev_c,
                             start=False, stop=True, skip_group_check=True)
            nc.scalar.copy(kq[:, tsl], pk)

        def v_mms(cx):
            tsl = slice(cx * CW, (cx + 1) * CW)
            pv = ps_proj.tile([128, CW], F32, tag="proj", name="pv")
            pq[("v", cx)] = pv
            pv64 = pv[0:HEAD_DIM, :]
            for dc in range(ND):
                nc.tensor.matmul(pv64, lhsT=wv[:, dc, :], rhs=xt[:, dc, tsl],
                                 start=(dc == 0), stop=(dc == ND - 1))

        def v_evict(cx):
            tsl = slice(cx * CW, (cx + 1) * CW)
            nc.vector.tensor_copy(vt[0:HEAD_DIM, tsl], pq[("v", cx)][0:HEAD_DIM, :])

        def vtrans(g0, g1):
            pvt = ps_rk.tile([128, CW], F32, tag="rk", name="pvt")
            for si in range(g0, g1):
                nc.tensor.transpose(
                    out=pvt[:, (si - g0) * vcols:(si - g0 + 1) * vcols],
                    in_=vt[:, si * 128:(si + 1) * 128],
                    identity=perm_sb[0:vcols, 3, 0:vcols].bitcast(F32),
                )
            nc.vector.tensor_copy(
                vn[:, g0 * vcols:g1 * vcols],
                pvt[:, 0:(g1 - g0) * vcols])

        groups = [(0, 5), (5, nt)] if nt > 5 else [(0, nt)]
        sched = [
            [lambda: qk_mms(0)],
            [lambda: ev_muls(0), lambda: qk_mms(1)],
            [lambda: rot_block(0)],
            [lambda: kq_block(0), lambda: ev_muls(1), lambda: qk_mms(2)],
            [lambda: rot_block(1)],
            [lambda: kq_block(1), lambda: ev_muls(2)],
            [lambda: rot_block(2)],
            [lambda: kq_block(2), lambda: v_mms(0)],
            [lambda: v_evict(0), lambda: v_mms(1)],
            [lambda: v_evict(1), lambda: v_mms(2)],
            [lambda: v_evict(2)],
        ] + [[lambda g0=g0, g1=g1: vtrans(g0, g1)] for g0, g1 in groups]
        for step in sched:
            for fn in step:
                fn()
            yield

    def make_finisher(idx, cx, att_sb, ra):
        b, h = pairs[idx]
        s = st[idx]
        wo = s["wo"]

        def fin():
            for k in range(TPC):
                ti = cx * TPC + k
                osb = outp.tile([128, HIDDEN], F16, tag="osb", name="osb")
                for dh in range(2):
                    po = ps_o.tile([128, 512], F32, tag="o", name="po")
                    nc.tensor.matmul(
                        po,
                        lhsT=att_sb[0:HEAD_DIM, k * 128:(k + 1) * 128],
                        rhs=wo[:, dh * 512:(dh + 1) * 512],
                        start=True, stop=True,
                    )
                    dst = osb[:, dh * 512:(dh + 1) * 512]
                    if (k * 2 + dh) % 2 == 0:
                        nc.scalar.mul(dst, po, ra[:, k:k + 1])
                    else:
                        nc.vector.tensor_scalar_mul(dst, po, ra[:, k:k + 1])
                    yield
                nc.sync.dma_start(
                    out=out_d[b, h, ti * 128:(ti + 1) * 128, :], in_=osb)
        return fin()

    def step_pending():
        if pending:
            if next(pending[0], StopIteration) is StopIteration:
                pending.pop(0)

    def drain_oldest():
        if pending:
            gen = pending.pop(0)
            for _ in gen:
                pass

    def emit_c_chunk(idx, cx, filler=None):
        s = st[idx]
        qkrot, kq, vn = s["qkrot"], s["kq"], s["vn"]
        tsl = slice(cx * CW, (cx + 1) * CW)
        n_s = TPC * (cx + 1)
        while len(pending) > 1:
            drain_oldest()
        patt = ps_att.tile([HEAD_DIM + 1, CW], F32, tag="att", name="patt")
        prob_tiles = []
        vcols = HEAD_DIM + 1

        def att_mm(si):
            kd = si - TPC * cx
            lo = kd * 128 if kd > 0 else 0
            nc.tensor.matmul(patt[:, lo:], lhsT=vn[:, si * vcols:(si + 1) * vcols],
                             rhs=prob_tiles[si][:, lo:],
                             start=(si == 0), stop=(si == n_s - 1),
                             skip_group_check=True)

        for si in range(n_s):
            psc = ps_sc.tile([128, CW], F32, tag="sc", name="psc")
            kd = si - TPC * cx
            diag = kd >= 0
            lo = 128 if kd >= 1 else 0
            tslo = slice(cx * CW + lo, (cx + 1) * CW)
            if si % 2 == 0:
                nc.tensor.matmul(
                    psc[:, lo:],
                    lhsT=kq[0:HEAD_DIM, si * 128:(si + 1) * 128],
                    rhs=qkrot[0:HEAD_DIM, tslo],
                    start=True, stop=not diag,
                )
            else:
                nc.tensor.matmul(
                    psc[:, lo:],
                    lhsT=qkrot[HEAD_DIM:128, si * 128:(si + 1) * 128],
                    rhs=kq[HEAD_DIM:128, tslo],
                    start=True, stop=not diag,
                )
            if diag:
                nc.tensor.matmul(
                    psc[:, kd * 128:(kd + 1) * 128],
                    lhsT=tri_sb[:, 0, :], rhs=tri_sb[:, 1, :],
                    start=False, stop=True, skip_group_check=True,
                )
            pt = ptp.tile([128, CW], BF16, tag="pt", name="pt")
            if diag and kd > 0:
                # columns left of the diagonal block are fully masked (s > t):
                # zero them on the (otherwise idle) Pool engine and exp the rest
                nc.gpsimd.memset(pt[:, 0:kd * 128], 0.0)
                nc.scalar.activation(pt[:, kd * 128:], psc[:, kd * 128:],
                                     mybir.ActivationFunctionType.Exp)
            else:
                nc.scalar.activation(pt, psc, mybir.ActivationFunctionType.Exp)
            prob_tiles.append(pt)
            if filler is not None:
                next(filler, None)
            step_pending()
            if si >= 2:
                att_mm(si - 2)
        att_mm(n_s - 2)
        att_mm(n_s - 1)

        att_sb = attp.tile([HEAD_DIM + 1, CW], F32R, tag="attsb", name="att_sb")
        nc.scalar.copy(att_sb, patt)
        # denominators: PE-transpose row 64 to columns, tiny DVE reciprocal
        pdn = ps_proj.tile([128, CW], F32, tag="proj", name="pdn")
        for k in range(TPC):
            nc.tensor.transpose(
                out=pdn[:, k:k + 1],
                in_=att_sb[HEAD_DIM:HEAD_DIM + 1,
                           k * 128:(k + 1) * 128].bitcast(F32),
                identity=ones_sb[HEAD_DIM:HEAD_DIM + 1, :],
            )
        ra = rap.tile([128, TPC], F32, tag="ra", name="ra")
        nc.vector.reciprocal(ra, pdn[:, 0:TPC])
        pending.append(make_finisher(idx, cx, att_sb, ra))

    # ---------- interleaved pipeline across pairs ----------
    emit_tables(0)
    for _ in emit_b_steps(0):
        pass
    for idx in range(NPAIRS):
        filler = None
        if idx + 1 < NPAIRS:
            emit_tables(idx + 1)
            filler = emit_b_steps(idx + 1)
        for cx in range(nchunks):
            emit_c_chunk(idx, cx, filler)
        if filler is not None:
            for _ in filler:
                pass
        if idx > 0:
            del st[idx - 1]
    while pending:
        drain_oldest()


_PROGRAM = {}


def _prep_in_maps(inputs):
    xt, cs, a01, wqk16, wv16, wo32, perms, tri, meta = _host_prep(inputs)
    in_maps = []
    for c in range(NCORES):
        hs = slice(c * HPC, (c + 1) * HPC)
        in_maps.append({
            "xt": np.ascontiguousarray(xt[:, hs]),
            "cs": np.ascontiguousarray(cs[:, hs]),
            "a01": np.ascontiguousarray(a01[:, hs]),
            "wqk": np.ascontiguousarray(wqk16[hs]),
            "wv": np.ascontiguousarray(wv16[hs]),
            "wo": np.ascontiguousarray(wo32[hs]),
            "perm": perms,
            "tri": tri,
        })
    return in_maps, meta


def kernel(**inputs) -> np.ndarray:
    in_maps, meta = _prep_in_maps(inputs)
    na = meta["na"]

    if na not in _PROGRAM:
        _PROGRAM[na] = _build_program(na)
    nc = _PROGRAM[na]

    res = run_bass_kernel_spmd(nc, in_maps, list(range(NCORES)))

    out = np.zeros((B, HEADS, T, HIDDEN), dtype=np.float32)
    idx = meta["idx"]
    for c in range(NCORES):
        oc = res.results[c]["out"]  # [B, HPC, na, HIDDEN] fp16
        for b in range(B):
            for hh in range(HPC):
                l = c * HPC + hh
                ii = idx[b][l]
                out[b, l, ii, :] = oc[b, hh, :len(ii), :].astype(np.float32)
    return out


# revision 30
# speedup vs baseline: 1.0098x; 1.0098x over previous
"""Trainium2 Bass kernel for BottleneckedEnsembleAttention (sparse/compacted).

Sharding: 8 cores, core c handles heads [2c, 2c+1] for both batches
(4 independent (b, head) attention problems per core).

Sparsity: the reference zeroes output rows for inactive queries, masks
inactive keys out of the softmax, and inactive tokens never otherwise
contribute.  The host therefore COMPACTS each (b, h) problem to its active
tokens (order-preserving, so the causal mask stays lower-triangular), pads
to NA = ceil(max_active/128)*128, and scatters the device output back into
a zero tensor.  Seed-0 counts are ~1024 of 2048, so NA = 1152: projections
shrink ~2x and attention area ~3.2x.

Host also pre-transposes the compacted X to [HIDDEN, NA] fp16 (no on-device
transposes for X), folds the softmax scale into Wq, and computes compacted
YaRN cos/sin tables [32, NA] fp16 (rows are 32-periodic on device).

Per (b, h) on-device pipeline (all heavy matmuls 1 cycle/col):
  1. qk pass: psum_qk = [Wq*scale | Wk]^T X^T  -> [128, ch] (q^T rows 0-63,
     k^T rows 64-127), fp16 inputs.
  2. RoPE on PE: ev_c = psum_qk * cos, ev_s = psum_qk * sin (DVE);
     cos/sin rows are 32-periodic so rotate-half/q-k-swap permutations
     commute with the elementwise multiplies:
       qkrot = P_rot @ ev_s + ev_c          (2 matmuls into one psum)
       kq    = (Psw P_rot) @ ev_s + Psw @ ev_c   (k in rows 0-63, q in 64-127)
  3. v^T pass (fp16) -> vt [65, NA] with row 64 = active-indicator (for the
     free softmax denominator), PE-transposed to vn [s, 65].
  4. per t-chunk (384 cols): scores^T[s-tile, t] = k lhsT @ q rhs; causal
     mask added INSIDE the matmul via an fp16 strict-upper -60000 triangular
     lhsT against an identity rhs on the diagonal 128-block; exp via ACT;
     att^T[u, t] accumulated over s-tiles (row 64 = denominator);
     o_proj: out[t-tile, 1024] = att^T lhsT @ Wo rhs, scaled by 1/denom at
     PSUM eviction (denom reciprocal as a row, PE-transposed to columns).
  5. store compacted fp16 output rows; host upcasts and scatters.

The emission is software-pipelined across the 4 (b, h) problems: the next
pair's projections/RoPE/v are interleaved as PE gap-filler inside the
current pair's attention chunks, and each chunk's o_proj/store is deferred
one chunk so the PE never waits on eviction chains.  PSUM evictions are
spread across ACT/DVE/Pool engines.
"""

import math
from contextlib import ExitStack

import numpy as np

import concourse.bass as bass
import concourse.mybir as mybir
import concourse.tile as tile
from concourse import bacc
from concourse.bass_utils import run_bass_kernel_spmd

# model constants (must match reference.py)
HIDDEN = 1024
HEADS = 16
HEAD_DIM = 64
THETA = 10000.0
TRAIN_LEN = 2048
SCALE = 4.0
ALPHA = 1.0
BETA = 32.0
B, T = 2, 2048

NCORES = 8
HPC = HEADS // NCORES  # heads per core = 2
NPAIRS = B * HPC       # independent (b, h) problems per core = 4

F32 = mybir.dt.float32
F32R = mybir.dt.float32r
F16 = mybir.dt.float16
BF16 = mybir.dt.bfloat16

ND = HIDDEN // 128  # 8 d-chunks
CW = 384            # chunk width (3 t-tiles)
TPC = CW // 128     # t-tiles per chunk = 3
NEG_TRI = -60000.0  # fp16-representable; exp(score + NEG_TRI) == 0.0


def _yarn_inv_freq():
    half = HEAD_DIM // 2
    pos_freqs = THETA ** (np.arange(half, dtype=np.float32) * 2.0 / HEAD_DIM)
    inv_freq_extra = (1.0 / pos_freqs).astype(np.float32)
    inv_freq_inter = (1.0 / (SCALE * pos_freqs)).astype(np.float32)

    def find_dim(num_rot):
        return (HEAD_DIM * math.log(TRAIN_LEN / (num_rot * 2.0 * math.pi))) / (
            2.0 * math.log(THETA)
        )

    low = max(math.floor(find_dim(BETA)), 0)
    high = min(math.ceil(find_dim(ALPHA)), half - 1)
    ramp = np.clip(
        (np.arange(half, dtype=np.float32) - low) / max(high - low, 1e-3), 0.0, 1.0
    ).astype(np.float32)
    extrap = (1.0 - ramp).astype(np.float32)
    inv_freq = inv_freq_inter * (1.0 - extrap) + inv_freq_extra * extrap
    mscale = 0.1 * math.log(SCALE) + 1.0 if SCALE > 1.0 else 1.0
    return inv_freq.astype(np.float32), np.float32(mscale)


def _perm_consts():
    """Permutation lhsT matrices for RoPE on the PE.

    P_rot: within each 64-row block (q rows 0-63, k rows 64-127),
      (P v)[u] = -v[u+32] for u<32, +v[u-32] for u>=32  (rotate-half w/ sign)
    P_swap: (P v)[u] = v[(u+64) % 128]                  (q<->k block swap)
    Matmul computes lhsT.T @ rhs, so pass the TRANSPOSE of each matrix.
    """
    P_rot = np.zeros((128, 128), dtype=np.float32)
    for blk in (0, 64):
        for u in range(32):
            P_rot[blk + u, blk + u + 32] = -1.0
            P_rot[blk + u + 32, blk + u] = 1.0
    P_swap = np.zeros((128, 128), dtype=np.float32)
    for u in range(128):
        P_swap[u, (u + 64) % 128] = 1.0
    P_swrot = P_swap @ P_rot
    ident = np.eye(128, dtype=np.float32)
    ones = np.ones((128, 128), dtype=np.float32)
    # [5, 128, 128]: lhsT variants (transposed), identity, all-ones
    perms = np.stack(
        [P_rot.T, P_swrot.T, P_swap.T, ident, ones], axis=0
    )
    return np.ascontiguousarray(perms)


def _tri_consts():
    """fp16 [3, 128, 128]: slot 0 = M^T where M[s,t] = NEG_TRI for s > t
    (strict lower triangle in (s, t)), slot 1 = identity, slot 2 = all
    NEG_TRI (for fully-masked s>t blocks left of the diagonal)."""
    M = np.tril(np.full((128, 128), NEG_TRI, dtype=np.float32), k=-1)
    full = np.full((128, 128), NEG_TRI, dtype=np.float32)
    out = np.stack([M.T, np.eye(128, dtype=np.float32), full], axis=0)
    return np.ascontiguousarray(out.astype(np.float16))


def _host_prep(inputs):
    x = np.asarray(inputs["packed_embeddings"], dtype=np.float32)
    pos = np.asarray(inputs["position_ids"])
    act = np.asarray(inputs["active_mask"])
    wq = np.asarray(inputs["q_proj"], dtype=np.float32)
    wk = np.asarray(inputs["k_proj"], dtype=np.float32)
    wv = np.asarray(inputs["v_proj"], dtype=np.float32)
    wo = np.asarray(inputs["o_proj"], dtype=np.float32)

    inv_freq, mscale = _yarn_inv_freq()
    scale = np.float32(mscale / math.sqrt(HEAD_DIM))

    counts = act.sum(axis=-1)  # (B, HEADS)
    nt_act = max(1, int(-(-counts.max() // 128)))
    na = nt_act * 128
    # round tiles up to a multiple of TPC so chunks are uniform
    nt_act = -(-nt_act // TPC) * TPC
    na = nt_act * 128

    idx = [[np.nonzero(act[b, l])[0] for l in range(HEADS)] for b in range(B)]

    xt = np.zeros((B, HEADS, HIDDEN, na), dtype=np.float16)
    cs = np.zeros((B, HEADS, 2, HEAD_DIM // 2, na), dtype=np.float16)
    a01 = np.zeros((B, HEADS, 1, na), dtype=np.float32)
    for b in range(B):
        for l in range(HEADS):
            ii = idx[b][l]
            n = len(ii)
            xt[b, l, :, :n] = x[b, l, ii, :].T
            ang = pos[b, l, ii].astype(np.float32)[:, None] * inv_freq  # (n, 32)
            cs[b, l, 0, :, :n] = np.cos(ang).T
            cs[b, l, 1, :, :n] = np.sin(ang).T
            a01[b, l, 0, :n] = 1.0

    wqk = np.concatenate([wq * scale, wk, wv], axis=-1)  # (L, 1024, 192)
    wqk16 = np.ascontiguousarray(wqk.astype(np.float16))
    wv16 = None
    wo32 = np.ascontiguousarray(wo)

    perms = _perm_consts()
    tri = _tri_consts()
    meta = {"na": na, "nt_act": nt_act, "counts": counts, "idx": idx}
    return xt, cs, a01, wqk16, wv16, wo32, perms, tri, meta


def _build_program(na):
    nt = na // 128
    nc = bacc.Bacc("TRN2", target_bir_lowering=False, debug=False)

    xt_d = nc.declare_dram_parameter("xt", [B, HPC, HIDDEN, na], F16, isOutput=False)
    cs_d = nc.declare_dram_parameter("cs", [B, HPC, 2, HEAD_DIM // 2, na], F16,
                                     isOutput=False)
    a01_d = nc.declare_dram_parameter("a01", [B, HPC, 1, na], F32R, isOutput=False)
    wqk_d = nc.declare_dram_parameter("wqk", [HPC, HIDDEN, 192], F16, isOutput=False)
    wo_d = nc.declare_dram_parameter("wo", [HPC, HEAD_DIM, HIDDEN], F32R,
                                     isOutput=False)
    perm_d = nc.declare_dram_parameter("perm", [5, 128, 128], F32R, isOutput=False)
    tri_d = nc.declare_dram_parameter("tri", [3, 128, 128], F16, isOutput=False)
    out_d = nc.declare_dram_parameter("out", [B, HPC, na, HIDDEN], F16, isOutput=True)

    with ExitStack() as ctx:
        tc = ctx.enter_context(tile.TileContext(nc))
        _emit(ctx, tc, nc, na, nt, xt_d, cs_d, a01_d, wqk_d, wo_d,
              perm_d, tri_d, out_d)
    nc.compile()
    return nc


def _emit(ctx, tc, nc, na, nt, xt_d, cs_d, a01_d, wqk_d, wo_d,
          perm_d, tri_d, out_d):
    nchunks = nt // TPC

    # ---- pools ----
    consts = ctx.enter_context(tc.tile_pool(name="consts", bufs=1))
    wpool = ctx.enter_context(tc.tile_pool(name="wpool", bufs=2))
    xtp = ctx.enter_context(tc.tile_pool(name="xt", bufs=2))
    cssp = ctx.enter_context(tc.tile_pool(name="css", bufs=2))
    qkp = ctx.enter_context(tc.tile_pool(name="qk", bufs=2))
    evp = ctx.enter_context(tc.tile_pool(name="ev", bufs=2))
    vtp = ctx.enter_context(tc.tile_pool(name="vt", bufs=2))
    vnp = ctx.enter_context(tc.tile_pool(name="vn", bufs=2))
    ptp = ctx.enter_context(tc.tile_pool(name="pt", bufs=4))
    attp = ctx.enter_context(tc.tile_pool(name="att", bufs=2))
    rap = ctx.enter_context(tc.tile_pool(name="ra", bufs=2))
    outp = ctx.enter_context(tc.tile_pool(name="outsb", bufs=4))

    ps_proj = ctx.enter_context(tc.tile_pool(name="ps_proj", bufs=2, space="PSUM"))
    ps_rk = ctx.enter_context(tc.tile_pool(name="ps_rk", bufs=1, space="PSUM"))
    ps_sc = ctx.enter_context(tc.tile_pool(name="ps_sc", bufs=2, space="PSUM"))
    ps_att = ctx.enter_context(tc.tile_pool(name="ps_att", bufs=1, space="PSUM"))
    ps_o = ctx.enter_context(tc.tile_pool(name="ps_o", bufs=2, space="PSUM"))

    # ---- constants (once) ----
    perm_sb = consts.tile([128, 5, 128], F32R)
    nc.sync.dma_start(out=perm_sb, in_=perm_d.rearrange("k p m -> p k m"))
    tri_sb = consts.tile([128, 3, 128], F16)
    nc.sync.dma_start(out=tri_sb, in_=tri_d.rearrange("k p m -> p k m"))
    ones_sb = consts.tile([128, 1], F32)
    nc.vector.memset(ones_sb, 1.0)

    # PE p-state warmup: cheap back-to-back matmuls while DMAs stream
    warm = ps_o.tile([128, 512], F32, tag="o", name="warm")
    for _ in range(28):
        nc.tensor.matmul(warm[:, 0:128], lhsT=perm_sb[:, 3, :],
                         rhs=perm_sb[:, 3, :], start=True, stop=True,
                         skip_group_check=True)

    pairs = [(b, h) for b in range(B) for h in range(HPC)]
    st = {}       # per-pair state
    pending = []  # deferred chunk finishers

    # ---------- phase emitters ----------
    def emit_tables(idx):
        b, h = pairs[idx]
        s = st[idx] = {}
        t_qk = wpool.tile([128, ND, 192], F16, tag="wqk", name="t_qk")
        nc.sync.dma_start(out=t_qk, in_=wqk_d[h].rearrange("(c p) m -> p c m", p=128))
        s["xt"] = xtp.tile([128, ND, na], F16, tag="xt", name="xt_sb")
        src = xt_d[b, h].rearrange("(c p) t -> p c t", p=128)
        if idx == 0:
            qw = na // 4
            for j in range(4):
                csl = slice(j * qw, (j + 1) * qw)
                nc.sync.dma_start(out=s["xt"][:, :, csl], in_=src[:, :, csl])
        else:
            step = ND // 4
            for j in range(4):
                nc.sync.dma_start(out=s["xt"][:, j * step:(j + 1) * step, :],
                                  in_=src[:, j * step:(j + 1) * step, :])
        s["cos"] = cssp.tile([128, na], F16, tag="cos", name="cos_sb")
        base = cs_d[b, h, 0]
        nc.sync.dma_start(out=s["cos"], in_=bass.AP(
            tensor=base.tensor, offset=base.offset, ap=[[0, 4]] + list(base.ap)))
        s["sin"] = cssp.tile([128, na], F16, tag="sin", name="sin_sb")
        base = cs_d[b, h, 1]
        nc.sync.dma_start(out=s["sin"], in_=bass.AP(
            tensor=base.tensor, offset=base.offset, ap=[[0, 4]] + list(base.ap)))
        t_o = wpool.tile([HEAD_DIM, HIDDEN], F32R, tag="wo", name="t_o")
        nc.sync.dma_start(out=t_o, in_=wo_d[h])
        s["wqk"], s["wo"] = t_qk, t_o
        # vt with active-indicator row 64 (free softmax denominator)
        s["vt"] = vtp.tile([HEAD_DIM + 1, na], F32, tag="vt", name="vt_sb")
        nc.sync.dma_start(out=s["vt"][HEAD_DIM:HEAD_DIM + 1, :]
                          .bitcast(F32R), in_=a01_d[b, h])
        s["a01"] = s["vt"]

    def emit_b_steps(idx):
        # projections + RoPE + v for pair idx, software-pipelined across its
        # chunks so PE never waits on the DVE/ACT RoPE evicts.
        s = st[idx]
        xt, wqk = s["xt"], s["wqk"]
        cos, sin = s["cos"], s["sin"]
        qkrot = qkp.tile([128, na], F32R, tag="qkrot", name="qkrot")
        kq = qkp.tile([128, na], F32R, tag="kq", name="kq")
        s["qkrot"], s["kq"] = qkrot, kq
        vt = s["vt"]
        vcols = HEAD_DIM + 1
        vn = vnp.tile([128, nt * vcols], BF16, tag="vn", name="vn")
        s["vn"] = vn

        pq = {}
        ev = {}

        def qk_mms(cx):
            tsl = slice(cx * CW, (cx + 1) * CW)
            pq[cx] = ps_proj.tile([128, CW], F32, tag="proj", name="pq")
            for dc in range(ND):
                nc.tensor.matmul(pq[cx], lhsT=wqk[:, dc, 0:128],
                                 rhs=xt[:, dc, tsl],
                                 start=(dc == 0), stop=(dc == ND - 1))

        def ev_muls(cx):
            tsl = slice(cx * CW, (cx + 1) * CW)
            ev_c = evp.tile([128, CW], F32R, tag="evc", name="ev_c")
            nc.vector.tensor_mul(ev_c, pq[cx], cos[:, tsl])
            ev_s = evp.tile([128, CW], F32R, tag="evs", name="ev_s")
            nc.vector.tensor_mul(ev_s, pq[cx], sin[:, tsl])
            ev[cx] = (ev_c, ev_s)

        def rot_block(cx):
            tsl = slice(cx * CW, (cx + 1) * CW)
            ev_c, ev_s = ev[cx]
            pr = ps_rk.tile([128, CW], F32, tag="rk", name="pr")
            nc.tensor.matmul(pr, lhsT=perm_sb[:, 0, :], rhs=ev_s,
                             start=True, stop=False)
            nc.tensor.matmul(pr, lhsT=perm_sb[:, 3, :], rhs=ev_c,
                             start=False, stop=True, skip_group_check=True)
            nc.vector.tensor_copy(qkrot[:, tsl], pr)

        def kq_block(cx):
            tsl = slice(cx * CW, (cx + 1) * CW)
            ev_c, ev_s = ev[cx]
            pk = ps_rk.tile([128, CW], F32, tag="rk", name="pk")
            nc.tensor.matmul(pk, lhsT=perm_sb[:, 1, :], rhs=ev_s,
                             start=True, stop=False)
            nc.tensor.matmul(pk, lhsT=perm_sb[:, 2, :], rhs=ev_c,
                             start=False, stop=True, skip_group_check=True)
            nc.scalar.copy(kq[:, tsl], pk)

        def v_mms(cx):
            tsl = slice(cx * CW, (cx + 1) * CW)
            pv = ps_proj.tile([128, CW], F32, tag="proj", name="pv")
            pq[("v", cx)] = pv
            pv64 = pv[0:HEAD_DIM, :]
            for dc in range(ND):
                nc.tensor.matmul(pv64, lhsT=wqk[:, dc, 128:192],
                                 rhs=xt[:, dc, tsl],
                                 start=(dc == 0), stop=(dc == ND - 1))

        def v_evict(cx):
            tsl = slice(cx * CW, (cx + 1) * CW)
            nc.vector.tensor_copy(vt[0:HEAD_DIM, tsl], pq[("v", cx)][0:HEAD_DIM, :])

        def vtrans(g0, g1):
            pvt = ps_rk.tile([128, CW], F32, tag="rk", name="pvt")
            for si in range(g0, g1):
                nc.tensor.transpose(
                    out=pvt[:, (si - g0) * vcols:(si - g0 + 1) * vcols],
                    in_=vt[:, si * 128:(si + 1) * 128],
                    identity=perm_sb[0:vcols, 3, 0:vcols].bitcast(F32),
                )
            nc.vector.tensor_copy(
                vn[:, g0 * vcols:g1 * vcols],
                pvt[:, 0:(g1 - g0) * vcols])

        groups = [(0, 5), (5, nt)] if nt > 5 else [(0, nt)]
        sched = [
            [lambda: qk_mms(0)],
            [lambda: ev_muls(0), lambda: qk_mms(1)],
            [lambda: rot_block(0)],
            [lambda: kq_block(0), lambda: ev_muls(1), lambda: qk_mms(2)],
            [lambda: rot_block(1)],
            [lambda: kq_block(1), lambda: ev_muls(2)],
            [lambda: rot_block(2)],
            [lambda: kq_block(2), lambda: v_mms(0)],
            [lambda: v_evict(0), lambda: v_mms(1)],
            [lambda: v_evict(1), lambda: v_mms(2)],
            [lambda: v_evict(2)],
        ] + [[lambda g0=g0, g1=g1: vtrans(g0, g1)] for g0, g1 in groups]
        for step in sched:
            for fn in step:
                fn()
            yield

    def make_finisher(idx, cx, att_sb, ra):
        b, h = pairs[idx]
        s = st[idx]
        wo = s["wo"]

        def fin():
            for k in range(TPC):
                ti = cx * TPC + k
                osb = outp.tile([128, HIDDEN], F16, tag="osb", name="osb")
                for dh in range(2):
                    po = ps_o.tile([128, 512], F32, tag="o", name="po")
                    nc.tensor.matmul(
                        po,
                        lhsT=att_sb[0:HEAD_DIM, k * 128:(k + 1) * 128],
                        rhs=wo[:, dh * 512:(dh + 1) * 512],
                        start=True, stop=True,
                    )
                    dst = osb[:, dh * 512:(dh + 1) * 512]
                    if (k * 2 + dh) % 2 == 0:
                        nc.scalar.mul(dst, po, ra[:, k:k + 1])
                    else:
                        nc.vector.tensor_scalar_mul(dst, po, ra[:, k:k + 1])
                    yield
                nc.sync.dma_start(
                    out=out_d[b, h, ti * 128:(ti + 1) * 128, :], in_=osb)
        return fin()

    def step_pending():
        if pending:
            if next(pending[0], StopIteration) is StopIteration:
                pending.pop(0)

    def drain_oldest():
        if pending:
            gen = pending.pop(0)
            for _ in gen:
                pass

    def emit_c_chunk(idx, cx, filler=None):
        s = st[idx]
        qkrot, kq, vn = s["qkrot"], s["kq"], s["vn"]
        tsl = slice(cx * CW, (cx + 1) * CW)
        n_s = TPC * (cx + 1)
        while len(pending) > 1:
            drain_oldest()
        patt = ps_att.tile([HEAD_DIM + 1, CW], F32, tag="att", name="patt")
        prob_tiles = []
        vcols = HEAD_DIM + 1

        def att_mm(si):
            kd = si - TPC * cx
            lo = kd * 128 if kd > 0 else 0
            nc.tensor.matmul(patt[:, lo:], lhsT=vn[:, si * vcols:(si + 1) * vcols],
                             rhs=prob_tiles[si][:, lo:],
                             start=(si == 0), stop=(si == n_s - 1),
                             skip_group_check=True)

        for si in range(n_s):
            psc = ps_sc.tile([128, CW], F32, tag="sc", name="psc")
            kd = si - TPC * cx
            diag = kd >= 0
            lo = 128 if kd >= 1 else 0
            tslo = slice(cx * CW + lo, (cx + 1) * CW)
            if si % 2 == 0:
                nc.tensor.matmul(
                    psc[:, lo:],
                    lhsT=kq[0:HEAD_DIM, si * 128:(si + 1) * 128],
                    rhs=qkrot[0:HEAD_DIM, tslo],
                    start=True, stop=not diag,
                )
            else:
                nc.tensor.matmul(
                    psc[:, lo:],
                    lhsT=qkrot[HEAD_DIM:128, si * 128:(si + 1) * 128],
                    rhs=kq[HEAD_DIM:128, tslo],
                    start=True, stop=not diag,
                )
            if diag:
                nc.tensor.matmul(
                    psc[:, kd * 128:(kd + 1) * 128],
                    lhsT=tri_sb[:, 0, :], rhs=tri_sb[:, 1, :],
                    start=False, stop=True, skip_group_check=True,
                )
            pt = ptp.tile([128, CW], BF16, tag="pt", name="pt")
            if diag and kd > 0:
                # columns left of the diagonal block are fully masked (s > t):
                # zero them on the (otherwise idle) Pool engine and exp the rest
                nc.gpsimd.memset(pt[:, 0:kd * 128], 0.0)
                nc.scalar.activation(pt[:, kd * 128:], psc[:, kd * 128:],
                                     mybir.ActivationFunctionType.Exp)
            else:
                nc.scalar.activation(pt, psc, mybir.ActivationFunctionType.Exp)
            prob_tiles.append(pt)
            if filler is not None:
                next(filler, None)
            step_pending()
            if si >= 2:
                att_mm(si - 2)
        att_mm(n_s - 2)
        att_mm(n_s - 1)

        att_sb = attp.tile([HEAD_DIM + 1, CW], F32R, tag="attsb", name="att_sb")
        nc.scalar.copy(att_sb, patt)
        # denominators: PE-transpose row 64 to columns, tiny DVE reciprocal
        pdn = ps_proj.tile([128, CW], F32, tag="proj", name="pdn")
        for k in range(TPC):
            nc.tensor.transpose(
                out=pdn[:, k:k + 1],
                in_=att_sb[HEAD_DIM:HEAD_DIM + 1,
                           k * 128:(k + 1) * 128].bitcast(F32),
                identity=ones_sb[HEAD_DIM:HEAD_DIM + 1, :],
            )
        ra = rap.tile([128, TPC], F32, tag="ra", name="ra")
        nc.vector.reciprocal(ra, pdn[:, 0:TPC])
        pending.append(make_finisher(idx, cx, att_sb, ra))

    # ---------- interleaved pipeline across pairs ----------
    emit_tables(0)
    for _ in emit_b_steps(0):
        pass
    for idx in range(NPAIRS):
        filler = None
        if idx + 1 < NPAIRS:
            emit_tables(idx + 1)
            filler = emit_b_steps(idx + 1)
        for cx in range(nchunks):
            emit_c_chunk(idx, cx, filler)
        if filler is not None:
            for _ in filler:
                pass
        if idx > 0:
            del st[idx - 1]
    while pending:
        drain_oldest()


_PROGRAM = {}


def _prep_in_maps(inputs):
    xt, cs, a01, wqk16, wv16, wo32, perms, tri, meta = _host_prep(inputs)
    in_maps = []
    for c in range(NCORES):
        hs = slice(c * HPC, (c + 1) * HPC)
        in_maps.append({
            "xt": np.ascontiguousarray(xt[:, hs]),
            "cs": np.ascontiguousarray(cs[:, hs]),
            "a01": np.ascontiguousarray(a01[:, hs]),
            "wqk": np.ascontiguousarray(wqk16[hs]),
            "wo": np.ascontiguousarray(wo32[hs]),
            "perm": perms,
            "tri": tri,
        })
    return in_maps, meta


def kernel(**inputs) -> np.ndarray:
    in_maps, meta = _prep_in_maps(inputs)
    na = meta["na"]

    if na not in _PROGRAM:
        _PROGRAM[na] = _build_program(na)
    nc = _PROGRAM[na]

    res = run_bass_kernel_spmd(nc, in_maps, list(range(NCORES)))

    out = np.zeros((B, HEADS, T, HIDDEN), dtype=np.float32)
    idx = meta["idx"]
    for c in range(NCORES):
        oc = res.results[c]["out"]  # [B, HPC, na, HIDDEN] fp16
        for b in range(B):
            for hh in range(HPC):
                l = c * HPC + hh
                ii = idx[b][l]
                out[b, l, ii, :] = oc[b, hh, :len(ii), :].astype(np.float32)
    return out


# revision 31
# speedup vs baseline: 1.0487x; 1.0385x over previous
"""Trainium2 Bass kernel for BottleneckedEnsembleAttention (sparse/compacted).

Sharding: 8 cores, core c handles heads [2c, 2c+1] for both batches
(4 independent (b, head) attention problems per core).

Sparsity: the reference zeroes output rows for inactive queries, masks
inactive keys out of the softmax, and inactive tokens never otherwise
contribute.  The host therefore COMPACTS each (b, h) problem to its active
tokens (order-preserving, so the causal mask stays lower-triangular), pads
to NA = ceil(max_active/128)*128, and scatters the device output back into
a zero tensor.  Seed-0 counts are ~1024 of 2048, so NA = 1152: projections
shrink ~2x and attention area ~3.2x.

Host also pre-transposes the compacted X to [HIDDEN, NA] fp16 (no on-device
transposes for X), folds the softmax scale into Wq, and computes compacted
YaRN cos/sin tables [32, NA] fp16 (rows are 32-periodic on device).

Per (b, h) on-device pipeline (all heavy matmuls 1 cycle/col):
  1. qk pass: psum_qk = [Wq*scale | Wk]^T X^T  -> [128, ch] (q^T rows 0-63,
     k^T rows 64-127), fp16 inputs.
  2. RoPE on PE: ev_c = psum_qk * cos, ev_s = psum_qk * sin (DVE);
     cos/sin rows are 32-periodic so rotate-half/q-k-swap permutations
     commute with the elementwise multiplies:
       qkrot = P_rot @ ev_s + ev_c          (2 matmuls into one psum)
       kq    = (Psw P_rot) @ ev_s + Psw @ ev_c   (k in rows 0-63, q in 64-127)
  3. v^T pass (fp16) -> vt [65, NA] with row 64 = active-indicator (for the
     free softmax denominator), PE-transposed to vn [s, 65].
  4. per t-chunk (384 cols): scores^T[s-tile, t] = k lhsT @ q rhs; causal
     mask added INSIDE the matmul via an fp16 strict-upper -60000 triangular
     lhsT against an identity rhs on the diagonal 128-block; exp via ACT;
     att^T[u, t] accumulated over s-tiles (row 64 = denominator);
     o_proj: out[t-tile, 1024] = att^T lhsT @ Wo rhs, scaled by 1/denom at
     PSUM eviction (denom reciprocal as a row, PE-transposed to columns).
  5. store compacted fp16 output rows; host upcasts and scatters.

The emission is software-pipelined across the 4 (b, h) problems: the next
pair's projections/RoPE/v are interleaved as PE gap-filler inside the
current pair's attention chunks, and each chunk's o_proj/store is deferred
one chunk so the PE never waits on eviction chains.  PSUM evictions are
spread across ACT/DVE/Pool engines.
"""

import math
from contextlib import ExitStack

import numpy as np

import concourse.bass as bass
import concourse.mybir as mybir
import concourse.tile as tile
from concourse import bacc
from concourse.bass_utils import run_bass_kernel_spmd

# model constants (must match reference.py)
HIDDEN = 1024
HEADS = 16
HEAD_DIM = 64
THETA = 10000.0
TRAIN_LEN = 2048
SCALE = 4.0
ALPHA = 1.0
BETA = 32.0
B, T = 2, 2048

NCORES = 8
HPC = HEADS // NCORES  # heads per core = 2
NPAIRS = B * HPC       # independent (b, h) problems per core = 4

F32 = mybir.dt.float32
F32R = mybir.dt.float32r
F16 = mybir.dt.float16
BF16 = mybir.dt.bfloat16

ND = HIDDEN // 128  # 8 d-chunks
CW = 384            # chunk width (3 t-tiles)
TPC = CW // 128     # t-tiles per chunk = 3
NEG_TRI = -60000.0  # fp16-representable; exp(score + NEG_TRI) == 0.0


def _yarn_inv_freq():
    half = HEAD_DIM // 2
    pos_freqs = THETA ** (np.arange(half, dtype=np.float32) * 2.0 / HEAD_DIM)
    inv_freq_extra = (1.0 / pos_freqs).astype(np.float32)
    inv_freq_inter = (1.0 / (SCALE * pos_freqs)).astype(np.float32)

    def find_dim(num_rot):
        return (HEAD_DIM * math.log(TRAIN_LEN / (num_rot * 2.0 * math.pi))) / (
            2.0 * math.log(THETA)
        )

    low = max(math.floor(find_dim(BETA)), 0)
    high = min(math.ceil(find_dim(ALPHA)), half - 1)
    ramp = np.clip(
        (np.arange(half, dtype=np.float32) - low) / max(high - low, 1e-3), 0.0, 1.0
    ).astype(np.float32)
    extrap = (1.0 - ramp).astype(np.float32)
    inv_freq = inv_freq_inter * (1.0 - extrap) + inv_freq_extra * extrap
    mscale = 0.1 * math.log(SCALE) + 1.0 if SCALE > 1.0 else 1.0
    return inv_freq.astype(np.float32), np.float32(mscale)


def _perm_consts():
    """Permutation lhsT matrices for RoPE on the PE.

    P_rot: within each 64-row block (q rows 0-63, k rows 64-127),
      (P v)[u] = -v[u+32] for u<32, +v[u-32] for u>=32  (rotate-half w/ sign)
    P_swap: (P v)[u] = v[(u+64) % 128]                  (q<->k block swap)
    Matmul computes lhsT.T @ rhs, so pass the TRANSPOSE of each matrix.
    """
    P_rot = np.zeros((128, 128), dtype=np.float32)
    for blk in (0, 64):
        for u in range(32):
            P_rot[blk + u, blk + u + 32] = -1.0
            P_rot[blk + u + 32, blk + u] = 1.0
    P_swap = np.zeros((128, 128), dtype=np.float32)
    for u in range(128):
        P_swap[u, (u + 64) % 128] = 1.0
    P_swrot = P_swap @ P_rot
    ident = np.eye(128, dtype=np.float32)
    ones = np.ones((128, 128), dtype=np.float32)
    # [5, 128, 128]: lhsT variants (transposed), identity, all-ones
    perms = np.stack(
        [P_rot.T, P_swrot.T, P_swap.T, ident, ones], axis=0
    )
    return np.ascontiguousarray(perms)


def _tri_consts():
    """fp16 [3, 128, 128]: slot 0 = M^T where M[s,t] = NEG_TRI for s > t
    (strict lower triangle in (s, t)), slot 1 = identity, slot 2 = all
    NEG_TRI (for fully-masked s>t blocks left of the diagonal)."""
    M = np.tril(np.full((128, 128), NEG_TRI, dtype=np.float32), k=-1)
    full = np.full((128, 128), NEG_TRI, dtype=np.float32)
    out = np.stack([M.T, np.eye(128, dtype=np.float32), full], axis=0)
    return np.ascontiguousarray(out.astype(np.float16))


def _host_prep(inputs):
    x = np.asarray(inputs["packed_embeddings"], dtype=np.float32)
    pos = np.asarray(inputs["position_ids"])
    act = np.asarray(inputs["active_mask"])
    wq = np.asarray(inputs["q_proj"], dtype=np.float32)
    wk = np.asarray(inputs["k_proj"], dtype=np.float32)
    wv = np.asarray(inputs["v_proj"], dtype=np.float32)
    wo = np.asarray(inputs["o_proj"], dtype=np.float32)

    inv_freq, mscale = _yarn_inv_freq()
    scale = np.float32(mscale / math.sqrt(HEAD_DIM))

    counts = act.sum(axis=-1)  # (B, HEADS)
    nt_act = max(1, int(-(-counts.max() // 128)))
    na = nt_act * 128
    # round tiles up to a multiple of TPC so chunks are uniform
    nt_act = -(-nt_act // TPC) * TPC
    na = nt_act * 128

    idx = [[np.nonzero(act[b, l])[0] for l in range(HEADS)] for b in range(B)]

    xt = np.zeros((B, HEADS, HIDDEN, na), dtype=np.float16)
    cs = np.zeros((B, HEADS, 2, HEAD_DIM // 2, na), dtype=np.float16)
    a01 = np.zeros((B, HEADS, 1, na), dtype=np.float32)
    for b in range(B):
        for l in range(HEADS):
            ii = idx[b][l]
            n = len(ii)
            xt[b, l, :, :n] = x[b, l, ii, :].T
            ang = pos[b, l, ii].astype(np.float32)[:, None] * inv_freq  # (n, 32)
            cs[b, l, 0, :, :n] = np.cos(ang).T
            cs[b, l, 1, :, :n] = np.sin(ang).T
            a01[b, l, 0, :n] = 1.0

    wqk = np.concatenate(
        [wq * scale, wk, wv, np.zeros_like(wv)], axis=-1)  # (L, 1024, 256)
    wqk16 = np.ascontiguousarray(wqk.astype(np.float16))
    wv16 = None
    wo32 = np.ascontiguousarray(wo)

    perms = _perm_consts()
    tri = _tri_consts()
    meta = {"na": na, "nt_act": nt_act, "counts": counts, "idx": idx}
    return xt, cs, a01, wqk16, wv16, wo32, perms, tri, meta


def _build_program(na):
    nt = na // 128
    nc = bacc.Bacc("TRN2", target_bir_lowering=False, debug=False)

    xt_d = nc.declare_dram_parameter("xt", [B, HPC, HIDDEN, na], F16, isOutput=False)
    cs_d = nc.declare_dram_parameter("cs", [B, HPC, 2, HEAD_DIM // 2, na], F16,
                                     isOutput=False)
    a01_d = nc.declare_dram_parameter("a01", [B, HPC, 1, na], F32R, isOutput=False)
    wqk_d = nc.declare_dram_parameter("wqk", [HPC, HIDDEN, 256], F16, isOutput=False)
    wo_d = nc.declare_dram_parameter("wo", [HPC, HEAD_DIM, HIDDEN], F32R,
                                     isOutput=False)
    perm_d = nc.declare_dram_parameter("perm", [5, 128, 128], F32R, isOutput=False)
    tri_d = nc.declare_dram_parameter("tri", [3, 128, 128], F16, isOutput=False)
    out_d = nc.declare_dram_parameter("out", [B, HPC, na, HIDDEN], F16, isOutput=True)

    with ExitStack() as ctx:
        tc = ctx.enter_context(tile.TileContext(nc))
        _emit(ctx, tc, nc, na, nt, xt_d, cs_d, a01_d, wqk_d, wo_d,
              perm_d, tri_d, out_d)
    nc.compile()
    return nc


def _emit(ctx, tc, nc, na, nt, xt_d, cs_d, a01_d, wqk_d, wo_d,
          perm_d, tri_d, out_d):
    nchunks = nt // TPC

    # ---- pools ----
    consts = ctx.enter_context(tc.tile_pool(name="consts", bufs=1))
    wpool = ctx.enter_context(tc.tile_pool(name="wpool", bufs=2))
    xtp = ctx.enter_context(tc.tile_pool(name="xt", bufs=2))
    cssp = ctx.enter_context(tc.tile_pool(name="css", bufs=2))
    qkp = ctx.enter_context(tc.tile_pool(name="qk", bufs=2))
    evp = ctx.enter_context(tc.tile_pool(name="ev", bufs=2))
    vtp = ctx.enter_context(tc.tile_pool(name="vt", bufs=2))
    vnp = ctx.enter_context(tc.tile_pool(name="vn", bufs=2))
    ptp = ctx.enter_context(tc.tile_pool(name="pt", bufs=4))
    attp = ctx.enter_context(tc.tile_pool(name="att", bufs=2))
    rap = ctx.enter_context(tc.tile_pool(name="ra", bufs=2))
    outp = ctx.enter_context(tc.tile_pool(name="outsb", bufs=4))

    ps_proj = ctx.enter_context(tc.tile_pool(name="ps_proj", bufs=2, space="PSUM"))
    ps_rk = ctx.enter_context(tc.tile_pool(name="ps_rk", bufs=1, space="PSUM"))
    ps_sc = ctx.enter_context(tc.tile_pool(name="ps_sc", bufs=2, space="PSUM"))
    ps_att = ctx.enter_context(tc.tile_pool(name="ps_att", bufs=1, space="PSUM"))
    ps_o = ctx.enter_context(tc.tile_pool(name="ps_o", bufs=2, space="PSUM"))

    # ---- constants (once) ----
    perm_sb = consts.tile([128, 5, 128], F32R)
    nc.sync.dma_start(out=perm_sb, in_=perm_d.rearrange("k p m -> p k m"))
    tri_sb = consts.tile([128, 3, 128], F16)
    nc.sync.dma_start(out=tri_sb, in_=tri_d.rearrange("k p m -> p k m"))
    ones_sb = consts.tile([128, 1], F32)
    nc.vector.memset(ones_sb, 1.0)

    # PE p-state warmup: wide back-to-back matmuls while DMAs stream
    warm = ps_o.tile([128, 512], F32, tag="o", name="warm")
    for _ in range(7):
        nc.tensor.matmul(warm, lhsT=perm_sb[:, 3, :],
                         rhs=perm_sb[:, 0:4, :], start=True, stop=True,
                         skip_group_check=True)

    pairs = [(b, h) for b in range(B) for h in range(HPC)]
    st = {}       # per-pair state
    pending = []  # deferred chunk finishers

    # ---------- phase emitters ----------
    def emit_tables(idx):
        b, h = pairs[idx]
        s = st[idx] = {}
        t_qk = wpool.tile([128, ND, 256], F16, tag="wqk", name="t_qk")
        nc.sync.dma_start(out=t_qk, in_=wqk_d[h].rearrange("(c p) m -> p c m", p=128))
        s["xt"] = xtp.tile([128, ND, na], F16, tag="xt", name="xt_sb")
        src = xt_d[b, h].rearrange("(c p) t -> p c t", p=128)
        if idx == 0:
            nc.sync.dma_start(out=s["xt"][:, :, 0:CW], in_=src[:, :, 0:CW])
        s["cos"] = cssp.tile([128, na], F16, tag="cos", name="cos_sb")
        base = cs_d[b, h, 0]
        nc.sync.dma_start(out=s["cos"], in_=bass.AP(
            tensor=base.tensor, offset=base.offset, ap=[[0, 4]] + list(base.ap)))
        s["sin"] = cssp.tile([128, na], F16, tag="sin", name="sin_sb")
        base = cs_d[b, h, 1]
        nc.sync.dma_start(out=s["sin"], in_=bass.AP(
            tensor=base.tensor, offset=base.offset, ap=[[0, 4]] + list(base.ap)))
        if idx == 0:
            for cx in range(1, nchunks):
                csl = slice(cx * CW, (cx + 1) * CW)
                nc.sync.dma_start(out=s["xt"][:, :, csl], in_=src[:, :, csl])
        else:
            step = ND // 4
            for j in range(4):
                nc.sync.dma_start(out=s["xt"][:, j * step:(j + 1) * step, :],
                                  in_=src[:, j * step:(j + 1) * step, :])
        t_o = wpool.tile([HEAD_DIM, HIDDEN], F32R, tag="wo", name="t_o")
        nc.sync.dma_start(out=t_o, in_=wo_d[h])
        s["wqk"], s["wo"] = t_qk, t_o
        # vt with active-indicator row 64 (free softmax denominator)
        s["vt"] = vtp.tile([HEAD_DIM + 1, na], F32, tag="vt", name="vt_sb")
        nc.sync.dma_start(out=s["vt"][HEAD_DIM:HEAD_DIM + 1, :]
                          .bitcast(F32R), in_=a01_d[b, h])
        s["a01"] = s["vt"]

    def emit_b_steps(idx):
        # projections + RoPE + v for pair idx, software-pipelined across its
        # chunks so PE never waits on the DVE/ACT RoPE evicts.
        s = st[idx]
        xt, wqk = s["xt"], s["wqk"]
        cos, sin = s["cos"], s["sin"]
        qkrot = qkp.tile([128, na], F32R, tag="qkrot", name="qkrot")
        kq = qkp.tile([128, na], F32R, tag="kq", name="kq")
        s["qkrot"], s["kq"] = qkrot, kq
        vt = s["vt"]
        vcols = HEAD_DIM + 1
        vn = vnp.tile([128, nt * vcols], BF16, tag="vn", name="vn")
        s["vn"] = vn

        pq = {}
        ev = {}

        def qk_mms(cx):
            tsl = slice(cx * CW, (cx + 1) * CW)
            pq[cx] = ps_proj.tile([128, CW], F32, tag="proj", name="pq")
            for dc in range(ND):
                nc.tensor.matmul(pq[cx], lhsT=wqk[:, dc, 0:128],
                                 rhs=xt[:, dc, tsl],
                                 start=(dc == 0), stop=(dc == ND - 1))

        def ev_muls(cx):
            tsl = slice(cx * CW, (cx + 1) * CW)
            ev_c = evp.tile([128, CW], F32R, tag="evc", name="ev_c")
            nc.vector.tensor_mul(ev_c, pq[cx], cos[:, tsl])
            ev_s = evp.tile([128, CW], F32R, tag="evs", name="ev_s")
            nc.vector.tensor_mul(ev_s, pq[cx], sin[:, tsl])
            ev[cx] = (ev_c, ev_s)

        def rot_block(cx):
            tsl = slice(cx * CW, (cx + 1) * CW)
            ev_c, ev_s = ev[cx]
            pr = ps_rk.tile([128, CW], F32, tag="rk", name="pr")
            nc.tensor.matmul(pr, lhsT=perm_sb[:, 0, :], rhs=ev_s,
                             start=True, stop=False)
            nc.tensor.matmul(pr, lhsT=perm_sb[:, 3, :], rhs=ev_c,
                             start=False, stop=True, skip_group_check=True)
            nc.vector.tensor_copy(qkrot[:, tsl], pr)

        def kq_block(cx):
            tsl = slice(cx * CW, (cx + 1) * CW)
            ev_c, ev_s = ev[cx]
            pk = ps_rk.tile([128, CW], F32, tag="rk", name="pk")
            nc.tensor.matmul(pk, lhsT=perm_sb[:, 1, :], rhs=ev_s,
                             start=True, stop=False)
            nc.tensor.matmul(pk, lhsT=perm_sb[:, 2, :], rhs=ev_c,
                             start=False, stop=True, skip_group_check=True)
            nc.scalar.copy(kq[:, tsl], pk)

        def v_mms(cx):
            tsl = slice(cx * CW, (cx + 1) * CW)
            pv = ps_proj.tile([128, CW], F32, tag="proj", name="pv")
            pq[("v", cx)] = pv
            pv64 = pv[0:HEAD_DIM, :]
            for dc in range(ND):
                nc.tensor.matmul(pv64, lhsT=wqk[:, dc, 128:192],
                                 rhs=xt[:, dc, tsl],
                                 start=(dc == 0), stop=(dc == ND - 1))

        def v_evict(cx):
            tsl = slice(cx * CW, (cx + 1) * CW)
            nc.vector.tensor_copy(vt[0:HEAD_DIM, tsl], pq[("v", cx)][0:HEAD_DIM, :])

        def vtrans(g0, g1):
            pvt = ps_rk.tile([128, CW], F32, tag="rk", name="pvt")
            for si in range(g0, g1):
                nc.tensor.transpose(
                    out=pvt[:, (si - g0) * vcols:(si - g0 + 1) * vcols],
                    in_=vt[:, si * 128:(si + 1) * 128],
                    identity=perm_sb[0:vcols, 3, 0:vcols].bitcast(F32),
                )
            nc.vector.tensor_copy(
                vn[:, g0 * vcols:g1 * vcols],
                pvt[:, 0:(g1 - g0) * vcols])

        groups = [(0, 5), (5, nt)] if nt > 5 else [(0, nt)]
        sched = [
            [lambda: qk_mms(0)],
            [lambda: ev_muls(0), lambda: qk_mms(1)],
            [lambda: rot_block(0)],
            [lambda: kq_block(0), lambda: ev_muls(1), lambda: qk_mms(2)],
            [lambda: rot_block(1)],
            [lambda: kq_block(1), lambda: ev_muls(2)],
            [lambda: rot_block(2)],
            [lambda: kq_block(2), lambda: v_mms(0)],
            [lambda: v_evict(0), lambda: v_mms(1)],
            [lambda: v_evict(1), lambda: v_mms(2)],
            [lambda: v_evict(2)],
        ] + [[lambda g0=g0, g1=g1: vtrans(g0, g1)] for g0, g1 in groups]
        for step in sched:
            for fn in step:
                fn()
            yield

    def make_finisher(idx, cx, att_sb, ra):
        b, h = pairs[idx]
        s = st[idx]
        wo = s["wo"]

        def fin():
            for k in range(TPC):
                ti = cx * TPC + k
                osb = outp.tile([128, HIDDEN], F16, tag="osb", name="osb")
                for dh in range(2):
                    po = ps_o.tile([128, 512], F32, tag="o", name="po")
                    nc.tensor.matmul(
                        po,
                        lhsT=att_sb[0:HEAD_DIM, k * 128:(k + 1) * 128],
                        rhs=wo[:, dh * 512:(dh + 1) * 512],
                        start=True, stop=True,
                    )
                    dst = osb[:, dh * 512:(dh + 1) * 512]
                    if (k * 2 + dh) % 2 == 0:
                        nc.scalar.mul(dst, po, ra[:, k:k + 1])
                    else:
                        nc.vector.tensor_scalar_mul(dst, po, ra[:, k:k + 1])
                    yield
                nc.sync.dma_start(
                    out=out_d[b, h, ti * 128:(ti + 1) * 128, :], in_=osb)
        return fin()

    def step_pending():
        if pending:
            if next(pending[0], StopIteration) is StopIteration:
                pending.pop(0)

    def drain_oldest():
        if pending:
            gen = pending.pop(0)
            for _ in gen:
                pass

    def emit_c_chunk(idx, cx, filler=None):
        s = st[idx]
        qkrot, kq, vn = s["qkrot"], s["kq"], s["vn"]
        tsl = slice(cx * CW, (cx + 1) * CW)
        n_s = TPC * (cx + 1)
        while len(pending) > 1:
            drain_oldest()
        patt = ps_att.tile([HEAD_DIM + 1, CW], F32, tag="att", name="patt")
        prob_tiles = []
        vcols = HEAD_DIM + 1

        def att_mm(si):
            kd = si - TPC * cx
            lo = kd * 128 if kd > 0 else 0
            nc.tensor.matmul(patt[:, lo:], lhsT=vn[:, si * vcols:(si + 1) * vcols],
                             rhs=prob_tiles[si][:, lo:],
                             start=(si == 0), stop=(si == n_s - 1),
                             skip_group_check=True)

        for si in range(n_s):
            psc = ps_sc.tile([128, CW], F32, tag="sc", name="psc")
            kd = si - TPC * cx
            diag = kd >= 0
            lo = 128 if kd >= 1 else 0
            tslo = slice(cx * CW + lo, (cx + 1) * CW)
            if si % 2 == 0:
                nc.tensor.matmul(
                    psc[:, lo:],
                    lhsT=kq[0:HEAD_DIM, si * 128:(si + 1) * 128],
                    rhs=qkrot[0:HEAD_DIM, tslo],
                    start=True, stop=not diag,
                )
            else:
                nc.tensor.matmul(
                    psc[:, lo:],
                    lhsT=qkrot[HEAD_DIM:128, si * 128:(si + 1) * 128],
                    rhs=kq[HEAD_DIM:128, tslo],
                    start=True, stop=not diag,
                )
            if diag:
                nc.tensor.matmul(
                    psc[:, kd * 128:(kd + 1) * 128],
                    lhsT=tri_sb[:, 0, :], rhs=tri_sb[:, 1, :],
                    start=False, stop=True, skip_group_check=True,
                )
            pt = ptp.tile([128, CW], BF16, tag="pt", name="pt")
            if diag and kd > 0:
                # columns left of the diagonal block are fully masked (s > t):
                # zero them on the (otherwise idle) Pool engine and exp the rest
                nc.gpsimd.memset(pt[:, 0:kd * 128], 0.0)
                nc.scalar.activation(pt[:, kd * 128:], psc[:, kd * 128:],
                                     mybir.ActivationFunctionType.Exp)
            else:
                nc.scalar.activation(pt, psc, mybir.ActivationFunctionType.Exp)
            prob_tiles.append(pt)
            if filler is not None:
                next(filler, None)
            step_pending()
            if si >= 2:
                att_mm(si - 2)
        att_mm(n_s - 2)
        att_mm(n_s - 1)

        att_sb = attp.tile([HEAD_DIM + 1, CW], F32R, tag="attsb", name="att_sb")
        nc.scalar.copy(att_sb, patt)
        # denominators: PE-transpose row 64 to columns, tiny DVE reciprocal
        pdn = ps_proj.tile([128, CW], F32, tag="proj", name="pdn")
        for k in range(TPC):
            nc.tensor.transpose(
                out=pdn[:, k:k + 1],
                in_=att_sb[HEAD_DIM:HEAD_DIM + 1,
                           k * 128:(k + 1) * 128].bitcast(F32),
                identity=ones_sb[HEAD_DIM:HEAD_DIM + 1, :],
            )
        ra = rap.tile([128, TPC], F32, tag="ra", name="ra")
        nc.vector.reciprocal(ra, pdn[:, 0:TPC])
        pending.append(make_finisher(idx, cx, att_sb, ra))

    # ---------- interleaved pipeline across pairs ----------
    emit_tables(0)
    for _ in emit_b_steps(0):
        pass
    for idx in range(NPAIRS):
        filler = None
        if idx + 1 < NPAIRS:
            emit_tables(idx + 1)
            filler = emit_b_steps(idx + 1)
        for cx in range(nchunks):
            emit_c_chunk(idx, cx, filler)
        if filler is not None:
            for _ in filler:
                pass
        if idx > 0:
            del st[idx - 1]
    while pending:
        drain_oldest()


_PROGRAM = {}


def _prep_in_maps(inputs):
    xt, cs, a01, wqk16, wv16, wo32, perms, tri, meta = _host_prep(inputs)
    in_maps = []
    for c in range(NCORES):
        hs = slice(c * HPC, (c + 1) * HPC)
        in_maps.append({
            "xt": np.ascontiguousarray(xt[:, hs]),
            "cs": np.ascontiguousarray(cs[:, hs]),
            "a01": np.ascontiguousarray(a01[:, hs]),
            "wqk": np.ascontiguousarray(wqk16[hs]),
            "wo": np.ascontiguousarray(wo32[hs]),
            "perm": perms,
            "tri": tri,
        })
    return in_maps, meta


def kernel(**inputs) -> np.ndarray:
    in_maps, meta = _prep_in_maps(inputs)
    na = meta["na"]

    if na not in _PROGRAM:
        _PROGRAM[na] = _build_program(na)
    nc = _PROGRAM[na]

    res = run_bass_kernel_spmd(nc, in_maps, list(range(NCORES)))

    out = np.zeros((B, HEADS, T, HIDDEN), dtype=np.float32)
    idx = meta["idx"]
    for c in range(NCORES):
        oc = res.results[c]["out"]  # [B, HPC, na, HIDDEN] fp16
        for b in range(B):
            for hh in range(HPC):
                l = c * HPC + hh
                ii = idx[b][l]
                out[b, l, ii, :] = oc[b, hh, :len(ii), :].astype(np.float32)
    return out
